# revision 6
# baseline (speedup 1.0000x reference)
"""Trainium2 Bass kernel for windowed dual-branch sparse attention.

Shapes (hardcoded): x [4096 windows, 64 tokens, 192 ch], 6 heads x 32.
Sharding: windows data-parallel across 8 cores (512 windows/core).
Per-core layout: supertile = 2 windows = 128 token-partitions.
"""
import sys

sys.path.insert(0, "/opt/trn_rl_repo")
import numpy as np

WIN = 8
DIM = 192
HEADS = 6
HD = 32
BWIN = 4096
NTOK = 64
NCORES = 8
BPC = BWIN // NCORES      # windows per core
NSUP_FULL = BPC // 2      # supertiles per core

_BUILD_CACHE = {}


def _build(nsup, c_blend):
    import concourse.bacc as bacc
    import concourse.tile as tile
    import concourse.mybir as mybir

    dt = mybir.dt
    F32, BF16 = dt.float32, dt.bfloat16
    AF = mybir.ActivationFunctionType
    ALU = mybir.AluOpType

    nc = bacc.Bacc("TRN2", target_bir_lowering=False, debug=False,
                   num_devices=NCORES)

    xs = nc.dram_tensor("xs", [nsup, 128, DIM], F32, kind="ExternalInput")
    ys = nc.dram_tensor("ys", [nsup, 128, DIM], F32, kind="ExternalOutput")
    # bf16 weights, k-chunked on first axis later via slicing
    wq = nc.dram_tensor("wq", [DIM, DIM], BF16, kind="ExternalInput")
    wk = nc.dram_tensor("wk", [DIM, DIM], BF16, kind="ExternalInput")
    wv = nc.dram_tensor("wv", [DIM, DIM], BF16, kind="ExternalInput")
    wp = nc.dram_tensor("wp", [DIM, DIM], BF16, kind="ExternalInput")
    qbias = nc.dram_tensor("qbias", [32, HEADS * 128], F32, kind="ExternalInput")
    kbias = nc.dram_tensor("kbias", [32, HEADS * 128], F32, kind="ExternalInput")
    biasm = nc.dram_tensor("biasm", [128, HEADS * NTOK], F32, kind="ExternalInput")
    bvm = nc.dram_tensor("bvm", [64, 2 * DIM], F32, kind="ExternalInput")
    bpm = nc.dram_tensor("bpm", [128, DIM], F32, kind="ExternalInput")
    idf = nc.dram_tensor("idf", [128, 128], F32, kind="ExternalInput")
    idb = nc.dram_tensor("idb", [128, 128], BF16, kind="ExternalInput")

    with tile.TileContext(nc) as tc:
        with (
            tc.tile_pool(name="const", bufs=1) as cp,
            tc.tile_pool(name="work", bufs=3) as wkp,
            tc.tile_pool(name="ps", bufs=8, space="PSUM") as pp,
        ):
            # ---- persistent constants ----
            def ld(name, src, shape, dtype):
                t = cp.tile(shape, dtype, tag=name)
                nc.sync.dma_start(t[:], src)
                return t

            wq0 = ld("wq0", wq[0:128, :], [128, DIM], BF16)
            wq1 = ld("wq1", wq[128:DIM, :], [64, DIM], BF16)
            wk0 = ld("wk0", wk[0:128, :], [128, DIM], BF16)
            wk1 = ld("wk1", wk[128:DIM, :], [64, DIM], BF16)
            wv0 = ld("wv0", wv[0:128, :], [128, DIM], BF16)
            wv1 = ld("wv1", wv[128:DIM, :], [64, DIM], BF16)
            wp0 = ld("wp0", wp[0:128, :], [128, DIM], BF16)
            wp1 = ld("wp1", wp[128:DIM, :], [64, DIM], BF16)
            qbt = ld("qbt", qbias[:], [32, HEADS * 128], F32)
            kbt = ld("kbt", kbias[:], [32, HEADS * 128], F32)
            biast = ld("biast", biasm[:], [128, HEADS * NTOK], F32)
            bvt = ld("bvt", bvm[:], [64, 2 * DIM], F32)
            bpt = ld("bpt", bpm[:], [128, DIM], F32)
            idft = ld("idft", idf[:], [128, 128], F32)
            idbt = ld("idbt", idb[:], [128, 128], BF16)

            for i in range(nsup):
                # ---- load x supertile (2 windows, 128 token rows) ----
                x_sb = wkp.tile([128, DIM], F32, tag="x")
                nc.sync.dma_start(x_sb[:], xs[i])

                # ---- xT = transpose(x): [192ch, 128tok] in 2 chunks ----
                xt0_ps = pp.tile([128, 128], F32, tag="ps")
                nc.tensor.transpose(xt0_ps[:], x_sb[:, 0:128], idft[:])
                xt1_ps = pp.tile([64, 128], F32, tag="ps")
                nc.tensor.transpose(xt1_ps[:], x_sb[:, 128:DIM], idft[:])
                xt0 = wkp.tile([128, 128], BF16, tag="xt0")
                nc.scalar.copy(xt0[:], xt0_ps[:])
                xt1 = wkp.tile([64, 128], BF16, tag="xt1")
                nc.scalar.copy(xt1[:], xt1_ps[:])

                # ---- q,k per-head layout [32d, 6h*128tok] (all pos 0) ----
                def proj_hf(wgt0, wgt1, btile, tag):
                    psa = pp.tile([32, 4 * 128], F32, tag="ps")
                    psb = pp.tile([32, 2 * 128], F32, tag="ps")
                    for h in range(HEADS):
                        ps = psa if h < 4 else psb
                        fo = (h % 4) * 128 if h < 4 else (h - 4) * 128
                        nc.tensor.matmul(ps[:, fo:fo + 128],
                                         wgt0[:, h * 32:(h + 1) * 32], xt0[:],
                                         start=True, stop=False)
                        nc.tensor.matmul(ps[:, fo:fo + 128],
                                         wgt1[:, h * 32:(h + 1) * 32], xt1[:],
                                         start=False, stop=True)
                    sb = wkp.tile([32, HEADS * 128], BF16, tag=tag)
                    nc.scalar.copy(sb[:, 0:512], psa[:])
                    nc.vector.tensor_add(sb[:, 512:768], psb[:],
                                         btile[:, 512:768])
                    return sb

                qt_sb = proj_hf(wq0, wq1, qbt, "qt")
                kt_sb = proj_hf(wk0, wk1, kbt, "kt")

                # ---- v token-major per window: [64tok, 2win*192ch] ----
                v_ps = pp.tile([64, 2 * DIM], F32, tag="ps")
                for w in (0, 1):
                    sl = slice(w * 64, w * 64 + 64)
                    nc.tensor.matmul(v_ps[:, w * DIM:(w + 1) * DIM],
                                     xt0[:, sl], wv0[:], start=True, stop=False)
                    nc.tensor.matmul(v_ps[:, w * DIM:(w + 1) * DIM],
                                     xt1[:, sl], wv1[:], start=False, stop=True)
                v_sb = wkp.tile([64, 2 * DIM], BF16, tag="v")
                nc.vector.tensor_add(v_sb[:], v_ps[:], bvt[:])

                # ---- attention scores, per-window psum (all col-pos 0) ----
                attn_sb = wkp.tile([128, HEADS * NTOK], F32, tag="attn")
                for w in (0, 1):
                    ap_w = pp.tile([64, HEADS * NTOK], F32, tag="ps")
                    for h in range(HEADS):
                        cs = slice(h * 128 + w * 64, h * 128 + (w + 1) * 64)
                        nc.tensor.matmul(
                            ap_w[0:64, h * 64:(h + 1) * 64],
                            qt_sb[:, cs], kt_sb[:, cs],
                            start=True, stop=True)
                    nc.vector.tensor_add(attn_sb[w * 64:(w + 1) * 64, :],
                                         ap_w[:], biast[0:64, :])

                # ---- softmax (no max-sub; |attn| < ~1) + relu^2 blend ----
                exp_sb = wkp.tile([128, HEADS * NTOK], BF16, tag="exp")
                nc.scalar.activation(exp_sb[:], attn_sb[:], AF.Exp)
                s6 = wkp.tile([128, HEADS], F32, tag="s6")
                nc.vector.tensor_reduce(
                    s6[:], exp_sb[:].rearrange("p (h t) -> p h t", h=HEADS),
                    axis=mybir.AxisListType.X, op=ALU.add)
                srec = wkp.tile([128, HEADS], F32, tag="srec")
                nc.vector.reciprocal(srec[:], s6[:])
                srec_c = wkp.tile([128, HEADS], F32, tag="srec_c")
                nc.vector.tensor_scalar_mul(srec_c[:], srec[:], float(c_blend))
                r2 = wkp.tile([128, HEADS * NTOK], BF16, tag="r2")
                nc.vector.scalar_tensor_tensor(
                    r2[:], attn_sb[:], 0.0, attn_sb[:], ALU.max, ALU.mult)
                a_sb = wkp.tile([128, HEADS * NTOK], BF16, tag="a")
                for h in range(HEADS):
                    hs = slice(h * 64, (h + 1) * 64)
                    nc.vector.scalar_tensor_tensor(
                        a_sb[:, hs], exp_sb[:, hs], srec_c[:, h:h + 1],
                        r2[:, hs], ALU.mult, ALU.add)

                # ---- aT: per head transpose [128(2w tq), 64tk]->[64tk, 128] ----
                at_ps = pp.tile([64, HEADS * 128], BF16, tag="ps")
                for h in range(HEADS):
                    nc.tensor.transpose(
                        at_ps[:, h * 128:(h + 1) * 128],
                        a_sb[:, h * 64:(h + 1) * 64], idbt[:])
                at_sb = wkp.tile([64, HEADS * 128], BF16, tag="at")
                nc.scalar.copy(at_sb[:, 0:384], at_ps[:, 0:384])
                nc.vector.tensor_copy(at_sb[:, 384:768], at_ps[:, 384:768])

                # ---- out_win natural [64tq, 192] per window (all pos 0) ----
                ow_sb = wkp.tile([64, 2 * DIM], BF16, tag="ow")
                for w in (0, 1):
                    ow_ps = pp.tile([64, DIM], F32, tag="ps")
                    for h in range(HEADS):
                        nc.tensor.matmul(
                            ow_ps[0:64, h * 32:(h + 1) * 32],
                            at_sb[:, h * 128 + w * 64: h * 128 + (w + 1) * 64],
                            v_sb[:, w * DIM + h * 32: w * DIM + (h + 1) * 32],
                            start=True, stop=True)
                    eng = nc.scalar if w == 0 else nc.vector
                    if w == 0:
                        nc.scalar.copy(ow_sb[:, 0:DIM], ow_ps[:])
                    else:
                        nc.vector.tensor_copy(ow_sb[:, DIM:2 * DIM], ow_ps[:])
                # transpose back to channel-major otT chunks for final proj
                ot0_ps = pp.tile([128, 128], BF16, tag="ps")
                ot1_ps = pp.tile([64, 128], BF16, tag="ps")
                for w in (0, 1):
                    nc.tensor.transpose(
                        ot0_ps[:, w * 64:(w + 1) * 64],
                        ow_sb[:, w * DIM: w * DIM + 128], idbt[0:64, 0:64])
                    nc.tensor.transpose(
                        ot1_ps[:, w * 64:(w + 1) * 64],
                        ow_sb[:, w * DIM + 128: w * DIM + DIM], idbt[0:64, 0:64])
                ot0 = wkp.tile([128, 128], BF16, tag="ot0")
                nc.scalar.copy(ot0[:], ot0_ps[:])
                ot1 = wkp.tile([64, 128], BF16, tag="ot1")
                nc.vector.tensor_copy(ot1[:], ot1_ps[:])

                # ---- final projection: y[128tok, 192] ----
                f_ps = pp.tile([128, DIM], F32, tag="ps")
                nc.tensor.matmul(f_ps[:], ot0[:], wp0[:], start=True, stop=False)
                nc.tensor.matmul(f_ps[:], ot1[:], wp1[:], start=False, stop=True)
                y_sb = wkp.tile([128, DIM], F32, tag="y")
                nc.vector.tensor_add(y_sb[:], f_ps[:], bpt[:])
                nc.sync.dma_start(ys[i], y_sb[:])

    nc.finalize()
    return nc


def _get_nc(nsup, c_blend):
    key = (nsup, round(float(c_blend), 9))
    if key not in _BUILD_CACHE:
        _BUILD_CACHE[key] = _build(nsup, c_blend)
    return _BUILD_CACHE[key]


def _host_consts(Wq, bq, Wkv, bkv, bias_table, Wp, bp, w, rel_index):
    import ml_dtypes
    bf16 = ml_dtypes.bfloat16
    scale = HD ** -0.5
    we = np.exp(np.asarray(w, np.float64) - np.max(w))
    ww = (we / we.sum()).astype(np.float64)
    c_blend = float(ww[0] / ww[1])
    wq_e = (np.asarray(Wq, np.float64) * scale).astype(bf16)
    wk_e = np.asarray(Wkv[:, :DIM], bf16)
    wv_e = (np.asarray(Wkv[:, DIM:], np.float64) * ww[1]).astype(bf16)
    wp_e = np.asarray(Wp, bf16)
    def headbias(b):
        b = np.asarray(b, np.float32).reshape(HEADS, 32).T  # [32d, 6h]
        return np.repeat(b[:, :, None], 128, axis=2).reshape(32, HEADS * 128)
    qbias = headbias(np.asarray(bq, np.float64) * scale)
    kbias = headbias(bkv[:DIM])
    # bias_full[tq, tk, h] -> [tq, h, tk] -> tile over 2 windows
    bias_full = np.asarray(bias_table)[np.asarray(rel_index)]
    biasm = np.tile(
        bias_full.transpose(0, 2, 1).reshape(NTOK, HEADS * NTOK), (2, 1)
    ).astype(np.float32)
    bvm = np.tile((np.asarray(bkv[DIM:], np.float64) * ww[1]).astype(np.float32),
                  (64, 2)).astype(np.float32)
    bpm = np.tile(np.asarray(bp, np.float32), (128, 1))
    idf = np.eye(128, dtype=np.float32)
    idb = np.eye(128).astype(bf16)
    return dict(wq=wq_e, wk=wk_e, wv=wv_e, wp=wp_e, qbias=qbias, kbias=kbias, biasm=biasm,
                bvm=bvm, bpm=bpm, idf=idf, idb=idb), c_blend


def kernel(x, Wq, bq, Wkv, bkv, bias_table, Wp, bp, w, rel_index,
           nsup=NSUP_FULL, _trace=False):
    from concourse.bass_utils import run_bass_kernel_spmd

    consts, c_blend = _host_consts(Wq, bq, Wkv, bkv, bias_table, Wp, bp, w,
                                   rel_index)
    nc = _get_nc(nsup, c_blend)
    x = np.asarray(x, np.float32)
    nwin = nsup * 2
    in_maps = []
    for c in range(NCORES):
        xc = x[c * BPC: c * BPC + nwin].reshape(nsup, 128, DIM)
        in_maps.append({"xs": np.ascontiguousarray(xc), **consts})
    br = run_bass_kernel_spmd(nc, in_maps, core_ids=list(range(NCORES)),
                              trace=_trace)
    out = np.empty((BWIN, NTOK, DIM), np.float32)
    for c in range(NCORES):
        out[c * BPC: c * BPC + nwin] = br.results[c]["ys"].reshape(
            nwin, NTOK, DIM)
    if nwin < BPC:  # partial build (testing only): fill rest with zeros
        for c in range(NCORES):
            out[c * BPC + nwin:(c + 1) * BPC] = 0.0
    kernel._last = br
    return out


# revision 7
# speedup vs baseline: 1.4795x; 1.4795x over previous
"""Trainium2 Bass kernel for windowed dual-branch sparse attention.

Shapes (hardcoded): x [4096 windows, 64 tokens, 192 ch], 6 heads x 32.
Sharding: windows data-parallel across 8 cores (512 windows/core).
Per-core layout: supertile = 2 windows = 128 token-partitions.
"""
import sys

sys.path.insert(0, "/opt/trn_rl_repo")
import numpy as np

WIN = 8
DIM = 192
HEADS = 6
HD = 32
BWIN = 4096
NTOK = 64
NCORES = 8
BPC = BWIN // NCORES      # windows per core
NSUP_FULL = BPC // 2      # supertiles per core

_BUILD_CACHE = {}


def _build(nsup, c_blend):
    import concourse.bacc as bacc
    import concourse.tile as tile
    import concourse.mybir as mybir

    dt = mybir.dt
    F32, BF16 = dt.float32, dt.bfloat16
    AF = mybir.ActivationFunctionType
    ALU = mybir.AluOpType

    nc = bacc.Bacc("TRN2", target_bir_lowering=False, debug=False,
                   num_devices=NCORES)

    xs = nc.dram_tensor("xs", [nsup, 128, DIM], F32, kind="ExternalInput")
    ys = nc.dram_tensor("ys", [nsup, 128, DIM], F32, kind="ExternalOutput")
    # bf16 weights, k-chunked on first axis later via slicing
    wq = nc.dram_tensor("wq", [DIM, DIM], BF16, kind="ExternalInput")
    wk = nc.dram_tensor("wk", [DIM, DIM], BF16, kind="ExternalInput")
    wv = nc.dram_tensor("wv", [DIM, DIM], BF16, kind="ExternalInput")
    wp = nc.dram_tensor("wp", [DIM, DIM], BF16, kind="ExternalInput")
    qbias = nc.dram_tensor("qbias", [32, HEADS * 128], F32, kind="ExternalInput")
    kbias = nc.dram_tensor("kbias", [32, HEADS * 128], F32, kind="ExternalInput")
    biasm = nc.dram_tensor("biasm", [128, HEADS * NTOK], F32, kind="ExternalInput")
    bvm = nc.dram_tensor("bvm", [64, 2 * DIM], F32, kind="ExternalInput")
    bpm = nc.dram_tensor("bpm", [128, DIM], F32, kind="ExternalInput")
    idf = nc.dram_tensor("idf", [128, 128], F32, kind="ExternalInput")
    idb = nc.dram_tensor("idb", [128, 128], BF16, kind="ExternalInput")

    with tile.TileContext(nc) as tc:
        with (
            tc.tile_pool(name="const", bufs=1) as cp,
            tc.tile_pool(name="work", bufs=4) as wkp,
            tc.tile_pool(name="ps", bufs=8, space="PSUM") as pp,
        ):
            # ---- persistent constants ----
            def ld(name, src, shape, dtype):
                t = cp.tile(shape, dtype, tag=name)
                nc.sync.dma_start(t[:], src)
                return t

            wq0 = ld("wq0", wq[0:128, :], [128, DIM], BF16)
            wq1 = ld("wq1", wq[128:DIM, :], [64, DIM], BF16)
            wk0 = ld("wk0", wk[0:128, :], [128, DIM], BF16)
            wk1 = ld("wk1", wk[128:DIM, :], [64, DIM], BF16)
            wv0 = ld("wv0", wv[0:128, :], [128, DIM], BF16)
            wv1 = ld("wv1", wv[128:DIM, :], [64, DIM], BF16)
            wp0 = ld("wp0", wp[0:128, :], [128, DIM], BF16)
            wp1 = ld("wp1", wp[128:DIM, :], [64, DIM], BF16)
            qbt = ld("qbt", qbias[:], [32, HEADS * 128], F32)
            kbt = ld("kbt", kbias[:], [32, HEADS * 128], F32)
            biast = ld("biast", biasm[:], [128, HEADS * NTOK], F32)
            bvt = ld("bvt", bvm[:], [64, 2 * DIM], F32)
            bpt = ld("bpt", bpm[:], [128, DIM], F32)
            idft = ld("idft", idf[:], [128, 128], F32)
            idbt = ld("idbt", idb[:], [128, 128], BF16)

            for i in range(nsup):
                # ---- load x supertile (2 windows, 128 token rows) ----
                x_sb = wkp.tile([128, DIM], F32, tag="x")
                nc.sync.dma_start(x_sb[:], xs[i])

                # ---- xT = transpose(x): [192ch, 128tok] in 2 chunks ----
                xt0_ps = pp.tile([128, 128], F32, tag="ps")
                nc.tensor.transpose(xt0_ps[:], x_sb[:, 0:128], idft[:])
                xt1_ps = pp.tile([64, 128], F32, tag="ps")
                nc.tensor.transpose(xt1_ps[:], x_sb[:, 128:DIM], idft[:])
                xt0 = wkp.tile([128, 128], BF16, tag="xt0")
                nc.scalar.copy(xt0[:], xt0_ps[:])
                xt1 = wkp.tile([64, 128], BF16, tag="xt1")
                nc.scalar.copy(xt1[:], xt1_ps[:])

                # ---- q,k per-head layout [32d, 6h*128tok] (all pos 0) ----
                def proj_hf(wgt0, wgt1, btile, tag):
                    psa = pp.tile([32, 4 * 128], F32, tag="ps")
                    psb = pp.tile([32, 2 * 128], F32, tag="ps")
                    for h in range(HEADS):
                        ps = psa if h < 4 else psb
                        fo = (h % 4) * 128 if h < 4 else (h - 4) * 128
                        nc.tensor.matmul(ps[:, fo:fo + 128],
                                         wgt0[:, h * 32:(h + 1) * 32], xt0[:],
                                         start=True, stop=False)
                        nc.tensor.matmul(ps[:, fo:fo + 128],
                                         wgt1[:, h * 32:(h + 1) * 32], xt1[:],
                                         start=False, stop=True)
                    sb = wkp.tile([32, HEADS * 128], BF16, tag=tag)
                    nc.scalar.copy(sb[:, 0:512], psa[:])
                    nc.vector.tensor_add(sb[:, 512:768], psb[:],
                                         btile[:, 512:768])
                    return sb

                qt_sb = proj_hf(wq0, wq1, qbt, "qt")
                kt_sb = proj_hf(wk0, wk1, kbt, "kt")

                # ---- v token-major per window: [64tok, 2win*192ch] ----
                v_ps = pp.tile([64, 2 * DIM], F32, tag="ps")
                for w in (0, 1):
                    sl = slice(w * 64, w * 64 + 64)
                    nc.tensor.matmul(v_ps[:, w * DIM:(w + 1) * DIM],
                                     xt0[:, sl], wv0[:], start=True, stop=False)
                    nc.tensor.matmul(v_ps[:, w * DIM:(w + 1) * DIM],
                                     xt1[:, sl], wv1[:], start=False, stop=True)
                v_sb = wkp.tile([64, 2 * DIM], BF16, tag="v")
                nc.vector.tensor_add(v_sb[:], v_ps[:], bvt[:])

                # ---- attention scores, per-window psum (all col-pos 0) ----
                attn_sb = wkp.tile([128, HEADS * NTOK], F32, tag="attn")
                for w in (0, 1):
                    ap_w = pp.tile([64, HEADS * NTOK], F32, tag="ps")
                    for h in range(HEADS):
                        cs = slice(h * 128 + w * 64, h * 128 + (w + 1) * 64)
                        nc.tensor.matmul(
                            ap_w[0:64, h * 64:(h + 1) * 64],
                            qt_sb[:, cs], kt_sb[:, cs],
                            start=True, stop=True)
                    nc.vector.tensor_add(attn_sb[w * 64:(w + 1) * 64, :],
                                         ap_w[:], biast[0:64, :])

                # ---- softmax (no max-sub; |attn| < ~1) + relu^2 blend ----
                exp_sb = wkp.tile([128, HEADS * NTOK], BF16, tag="exp")
                nc.scalar.activation(exp_sb[:], attn_sb[:], AF.Exp)
                s6 = wkp.tile([128, HEADS], F32, tag="s6")
                nc.vector.tensor_reduce(
                    s6[:], exp_sb[:].rearrange("p (h t) -> p h t", h=HEADS),
                    axis=mybir.AxisListType.X, op=ALU.add)
                srec = wkp.tile([128, HEADS], F32, tag="srec")
                nc.vector.reciprocal(srec[:], s6[:])
                srec_c = wkp.tile([128, HEADS], F32, tag="srec_c")
                nc.vector.tensor_scalar_mul(srec_c[:], srec[:], float(c_blend))
                r2 = wkp.tile([128, HEADS * NTOK], BF16, tag="r2")
                nc.vector.scalar_tensor_tensor(
                    r2[:], attn_sb[:], 0.0, attn_sb[:], ALU.max, ALU.mult)
                a_sb = wkp.tile([128, HEADS * NTOK], BF16, tag="a")
                for h in range(HEADS):
                    hs = slice(h * 64, (h + 1) * 64)
                    nc.vector.scalar_tensor_tensor(
                        a_sb[:, hs], exp_sb[:, hs], srec_c[:, h:h + 1],
                        r2[:, hs], ALU.mult, ALU.add)

                # ---- aT: per head transpose [128(2w tq), 64tk]->[64tk, 128] ----
                at_ps = pp.tile([64, HEADS * 128], BF16, tag="ps")
                for h in range(HEADS):
                    nc.tensor.transpose(
                        at_ps[:, h * 128:(h + 1) * 128],
                        a_sb[:, h * 64:(h + 1) * 64], idbt[:])
                at_sb = wkp.tile([64, HEADS * 128], BF16, tag="at")
                nc.scalar.copy(at_sb[:, 0:384], at_ps[:, 0:384])
                nc.vector.tensor_copy(at_sb[:, 384:768], at_ps[:, 384:768])

                # ---- out_win natural [64tq, 192] per window (all pos 0) ----
                ow_sb = wkp.tile([64, 2 * DIM], BF16, tag="ow")
                for w in (0, 1):
                    ow_ps = pp.tile([64, DIM], F32, tag="ps")
                    for h in range(HEADS):
                        nc.tensor.matmul(
                            ow_ps[0:64, h * 32:(h + 1) * 32],
                            at_sb[:, h * 128 + w * 64: h * 128 + (w + 1) * 64],
                            v_sb[:, w * DIM + h * 32: w * DIM + (h + 1) * 32],
                            start=True, stop=True)
                    eng = nc.scalar if w == 0 else nc.vector
                    if w == 0:
                        nc.scalar.copy(ow_sb[:, 0:DIM], ow_ps[:])
                    else:
                        nc.vector.tensor_copy(ow_sb[:, DIM:2 * DIM], ow_ps[:])
                # transpose back to channel-major otT chunks for final proj
                ot0_ps = pp.tile([128, 128], BF16, tag="ps")
                ot1_ps = pp.tile([64, 128], BF16, tag="ps")
                for w in (0, 1):
                    nc.tensor.transpose(
                        ot0_ps[:, w * 64:(w + 1) * 64],
                        ow_sb[:, w * DIM: w * DIM + 128], idbt[0:64, 0:64])
                    nc.tensor.transpose(
                        ot1_ps[:, w * 64:(w + 1) * 64],
                        ow_sb[:, w * DIM + 128: w * DIM + DIM], idbt[0:64, 0:64])
                ot0 = wkp.tile([128, 128], BF16, tag="ot0")
                nc.scalar.copy(ot0[:], ot0_ps[:])
                ot1 = wkp.tile([64, 128], BF16, tag="ot1")
                nc.vector.tensor_copy(ot1[:], ot1_ps[:])

                # ---- final projection: y[128tok, 192] ----
                f_ps = pp.tile([128, DIM], F32, tag="ps")
                nc.tensor.matmul(f_ps[:], ot0[:], wp0[:], start=True, stop=False)
                nc.tensor.matmul(f_ps[:], ot1[:], wp1[:], start=False, stop=True)
                y_sb = wkp.tile([128, DIM], F32, tag="y")
                nc.vector.tensor_add(y_sb[:], f_ps[:], bpt[:])
                nc.sync.dma_start(ys[i], y_sb[:])

    nc.finalize()
    return nc


def _get_nc(nsup, c_blend):
    key = (nsup, round(float(c_blend), 9))
    if key not in _BUILD_CACHE:
        _BUILD_CACHE[key] = _build(nsup, c_blend)
    return _BUILD_CACHE[key]


def _host_consts(Wq, bq, Wkv, bkv, bias_table, Wp, bp, w, rel_index):
    import ml_dtypes
    bf16 = ml_dtypes.bfloat16
    scale = HD ** -0.5
    we = np.exp(np.asarray(w, np.float64) - np.max(w))
    ww = (we / we.sum()).astype(np.float64)
    c_blend = float(ww[0] / ww[1])
    wq_e = (np.asarray(Wq, np.float64) * scale).astype(bf16)
    wk_e = np.asarray(Wkv[:, :DIM], bf16)
    wv_e = (np.asarray(Wkv[:, DIM:], np.float64) * ww[1]).astype(bf16)
    wp_e = np.asarray(Wp, bf16)
    def headbias(b):
        b = np.asarray(b, np.float32).reshape(HEADS, 32).T  # [32d, 6h]
        return np.repeat(b[:, :, None], 128, axis=2).reshape(32, HEADS * 128)
    qbias = headbias(np.asarray(bq, np.float64) * scale)
    kbias = headbias(bkv[:DIM])
    # bias_full[tq, tk, h] -> [tq, h, tk] -> tile over 2 windows
    bias_full = np.asarray(bias_table)[np.asarray(rel_index)]
    biasm = np.tile(
        bias_full.transpose(0, 2, 1).reshape(NTOK, HEADS * NTOK), (2, 1)
    ).astype(np.float32)
    bvm = np.tile((np.asarray(bkv[DIM:], np.float64) * ww[1]).astype(np.float32),
                  (64, 2)).astype(np.float32)
    bpm = np.tile(np.asarray(bp, np.float32), (128, 1))
    idf = np.eye(128, dtype=np.float32)
    idb = np.eye(128).astype(bf16)
    return dict(wq=wq_e, wk=wk_e, wv=wv_e, wp=wp_e, qbias=qbias, kbias=kbias, biasm=biasm,
                bvm=bvm, bpm=bpm, idf=idf, idb=idb), c_blend


def kernel(x, Wq, bq, Wkv, bkv, bias_table, Wp, bp, w, rel_index,
           nsup=NSUP_FULL, _trace=False):
    from concourse.bass_utils import run_bass_kernel_spmd

    consts, c_blend = _host_consts(Wq, bq, Wkv, bkv, bias_table, Wp, bp, w,
                                   rel_index)
    nc = _get_nc(nsup, c_blend)
    x = np.asarray(x, np.float32)
    nwin = nsup * 2
    in_maps = []
    for c in range(NCORES):
        xc = x[c * BPC: c * BPC + nwin].reshape(nsup, 128, DIM)
        in_maps.append({"xs": np.ascontiguousarray(xc), **consts})
    br = run_bass_kernel_spmd(nc, in_maps, core_ids=list(range(NCORES)),
                              trace=_trace)
    out = np.empty((BWIN, NTOK, DIM), np.float32)
    for c in range(NCORES):
        out[c * BPC: c * BPC + nwin] = br.results[c]["ys"].reshape(
            nwin, NTOK, DIM)
    if nwin < BPC:  # partial build (testing only): fill rest with zeros
        for c in range(NCORES):
            out[c * BPC + nwin:(c + 1) * BPC] = 0.0
    kernel._last = br
    return out
